# revision 59
# baseline (speedup 1.0000x reference)
"""Trainium2 Bass kernel for nn_ContrastiveLoss (SimCLR-style NT-Xent).

Reference computation:
    f = normalize(concat([z1, z2]))            # [2B, D] unit rows
    S = f @ f.T / T                            # [8192, 8192]
    loss = mean_i( logsumexp_j(S[i, :]) - S[i, pos_i] )

Symmetric sharding: S is symmetric, so each of the 8 cores computes only
5 of the 8 column-groups of its 1024-row block (groups 0..4 after
rotating the row-groups so the core's own rows are group 0).  The
missing column groups 5,6,7 of row-block b are transposes of blocks
computed on cores b-3..b-1 and are recovered as COLUMN sums of the
exp'd blocks g=1..3 (a [128, 2, 512] fp8 DoubleRow ones-matmul per
row-tile pair, accumulated in PSUM), exchanged between cores by the
host during the final cheap f64 reduction.  This cuts matmul + exp work
to 5/8 and HBM traffic to 10 MB/core.  On top of that, phases 0 and 4
are TRIANGULAR: g0 is the symmetric diagonal block and the g4 pair
block is otherwise computed by both cores of a +4 pair, so the
(r>=4, ns0) quarter of each is skipped and recovered from column sums
of the computed (r<4, ns1) quarter (own-core for g0, +4-partner for
g4; the r<4 exps split into a bf16-scratch ns0 half and an fp8 ns1
half feeding a [128, 2, 512] DR ones-matmul, with the ns1 row-sums
landing in 8 extra sums columns).

Operand layout: rows are normalized in row-major bf16 (DVE
affine_mul_reduce sum-of-squares + Quake rsqrt + scale), DMA-xbar
transposed as native 2-byte elements into [dp, db, col] (d = 128*db +
dp, one transpose per row-tile to stay within the 2D-in/3D-out xbar
constraint), then cast to fp8e4 per column half.  A DoubleRow
contraction pair (dp, t) maps to d = 256h + 128t + dp, so BOTH operands
slice straight out of the same [128, 4, 1024] fp8 tile with far-strided
(1024B) k-pairs and contiguous columns - the layout the double-pumped
weight/ifmap streams require (byte-interleaved pairs run 1 elem/cycle).

Pipeline (measured-best arrangement): loads are SWDGE f32->bf16
cast-DMAs (~150 GB/s each), paced two-wide by chaining chunk n behind
chunk n-2; group g+2's ssq mul-reduces are drip-fed one per row-tile
through phase g's DVE queue, and its rsqrt/scale+transpose/cast tail is
emitted at phase end; startup transposes for groups 0/1 are split over
both HWDGE queues (SP + ACT) while ACT is idle.  Per row-tile r, phase
g: 4 DR matmuls -> [128, 1024] psum; diag (g=0) / pos-pair (g=4) raw
cosines are extracted pre-exp from PSUM with an eye mul-reduce; ACT
exps the block with a fused row-sum (accum_out), writing bf16 scratch
(g=0/4) or fp8 for the colsum matmuls (g=1..3).
NOTE: tensor_tensor_reduce hangs TRN2 hardware (sim is fine) - all
mul-reduces must use affine_mul_reduce.

Host (f64) assembles denominators across cores:
  den[b] = rowsums_b - exp(diag_b/T) + e^{1/T} + sum_g colsums_{b-g}[g]
  loss   = mean(log(den) - pos/T)
The exact-diagonal substitution cancels the fp8 quantization noise of
the dominant e^{1/T} ~ 1.6e6 softmax term (the rest of a row sums to
~1e4).  Off-diagonal cosines are within ~+-0.25 whp, so exp(S/T) fits
fp8e4 directly for the colsum operands.  No logsumexp max-subtraction
is needed: sum_j exp() <= ~2e10 fits fp32.

Measured on TRN2 (8 cores): 132909-133434 ns in a heat-degraded
session window where the pre-triangular config measured 147-155 us
(fresh-device best of that config: 130352 ns), vs the 207-222 us v1
baseline; rel err 4.1e-5.  Alternatives measured
SLOWER and reverted: load lookahead-3 with up-front prep blobs (+8us),
scale-on-ACT (+10), per-phase output DMAs (+20), hard cross-group DVE
ordering deps (+15), ns-major warm-start matmuls (+20 combined), ssq
via ACT Square (+7, though it improves rel err to 7e-6).
"""

import os
import sys

# Reset the NeuronCores when the runtime opens the device: a prior run
# leaving the part in a degraded power/exec state costs 10-15% measured
# exec time (observed 133 -> 148 us on identical binaries); the reset
# restores the fast window.  setdefault so an explicit env still wins.
os.environ.setdefault("NEURON_RT_RESET_CORES", "1")

try:
    import concourse.bass  # noqa: F401
except ImportError:
    for _p in ("/root/.axon_site/_ro/trn_rl_repo", "/opt/trn_rl_repo"):
        if _p not in sys.path and os.path.isdir(_p):
            sys.path.insert(0, _p)

import numpy as np

B = 4096
D = 512
T = 0.07
P = 128
NCORES = 8
R = (2 * B) // NCORES
G = 8
NG = 5
GT = R // P
H = 2
DB = D // P

_NC = None


def _build():
    from contextlib import ExitStack

    import concourse.bacc as bacc
    import concourse.tile as tile
    from concourse import mybir
    from concourse.tile import add_dep_helper

    f32 = mybir.dt.float32
    bf16 = mybir.dt.bfloat16
    f8 = mybir.dt.float8e4
    i32 = mybir.dt.int32
    AFT = mybir.ActivationFunctionType
    EXPF = AFT.Exp
    MUL = mybir.AluOpType.mult
    ADD = mybir.AluOpType.add
    SUB = mybir.AluOpType.subtract
    SHR = mybir.AluOpType.logical_shift_right
    DR = mybir.MatmulPerfMode.DoubleRow

    nc = bacc.Bacc(
        "TRN2", target_bir_lowering=False, debug=False, num_devices=NCORES
    )
    fg = [
        nc.dram_tensor(f"f{k}", [R, D], f32, kind="ExternalInput")
        for k in range(NG)
    ]
    eye = nc.dram_tensor("eye", [P, P], f32, kind="ExternalInput")
    sums_out = nc.dram_tensor("sums", [P, NG * GT + 8], f32, kind="ExternalOutput")
    diag_out = nc.dram_tensor("diag", [P, GT], f32, kind="ExternalOutput")
    pos_out = nc.dram_tensor("pos", [P, GT], f32, kind="ExternalOutput")
    csum_out = nc.dram_tensor("csum", [1, 3 * R + 1024], f32, kind="ExternalOutput")

    with ExitStack() as ctx:
        tc = ctx.enter_context(tile.TileContext(nc))
        smalls = ctx.enter_context(tc.tile_pool(name="smalls", bufs=1))
        dumps = ctx.enter_context(tc.tile_pool(name="dumps", bufs=4))
        stats = ctx.enter_context(tc.tile_pool(name="stats", bufs=3))
        zbpool = ctx.enter_context(tc.tile_pool(name="zbpool", bufs=3))
        fnbpool = ctx.enter_context(tc.tile_pool(name="fnbpool", bufs=2))
        tbpool = ctx.enter_context(tc.tile_pool(name="tbpool", bufs=2))
        f8pool = ctx.enter_context(tc.tile_pool(name="f8pool", bufs=1))
        e8pool = ctx.enter_context(tc.tile_pool(name="e8pool", bufs=2))
        scrpool = ctx.enter_context(tc.tile_pool(name="scrpool", bufs=2))
        psum = ctx.enter_context(tc.tile_pool(name="psum", bufs=3, space="PSUM"))
        cspool = ctx.enter_context(tc.tile_pool(name="cspool", bufs=1, space="PSUM"))

        sums_sb = smalls.tile([P, NG * GT + 8], f32, tag="sums_sb")
        diag_sb = smalls.tile([P, GT], f32, tag="diag_sb")
        pos_sb = smalls.tile([P, GT], f32, tag="pos_sb")
        csum_sb = smalls.tile([1, 3 * R + 1024], f32, tag="csum_sb")
        eye_sb = smalls.tile([P, P], f32, tag="eye_sb")
        nc.sync.dma_start(out=eye_sb[:], in_=eye[:, :])
        magic = smalls.tile([P, GT], i32, tag="magic")
        nc.vector.memset(magic[:], 0x5F3759DF)
        ones8 = smalls.tile([P, 2, 16], f8, tag="ones8")
        nc.vector.memset(ones8[:], 1.0)

        def mulsum(in0, in1, accum_col):
            dummy = dumps.tile([P, 1], f32, tag="dummy")
            return nc.vector.affine_mul_reduce(
                out=dummy.broadcast_to(in0.shape),
                accum_out=accum_col,
                in0=in0,
                in1=in1,
                scale=1.0,
                bias=0.0,
            )

        def rsqrt(invn_dst, ssq):
            n = ssq.shape[1]
            h = stats.tile([P, n], i32, tag="h")
            nc.vector.tensor_scalar(h[:], ssq.bitcast(i32), 1, None, op0=SHR)
            y = stats.tile([P, n], f32, tag="y")
            nc.vector.tensor_tensor(y[:].bitcast(i32), magic[:, :n], h[:], op=SUB)
            a = stats.tile([P, n], f32, tag="a")
            for _ in range(2):
                nc.vector.tensor_mul(a[:], y[:], y[:])
                nc.vector.tensor_mul(a[:], a[:], ssq)
                nc.vector.tensor_scalar(a[:], a[:], -0.5, 1.5, op0=MUL, op1=ADD)
                nc.vector.tensor_mul(y[:], y[:], a[:])
            nc.vector.tensor_scalar_min(invn_dst, y[:], 1.0e12)

        load_insts = []
        zbs = {}

        def load_group(g):
            zb = zbpool.tile([P, GT, D], f32, tag="zb")
            for s in range(2):
                ld = nc.gpsimd.dma_start(
                    out=zb[:, s * 4 : (s + 1) * 4, :],
                    in_=fg[g][s * 4 * P : (s + 1) * 4 * P, :].rearrange(
                        "(a p) d -> p a d", p=P
                    ),
                )
                n = len(load_insts)
                if n >= 2:
                    add_dep_helper(
                        ld.ins, load_insts[n - 2].ins, reason="pace loads"
                    )
                load_insts.append(ld)
            zbs[g] = zb

        ft8s = {}
        ssqs = {}

        def prep_ssq(g, a):
            if g not in ssqs:
                ssqs[g] = stats.tile(
                    [P, GT], f32, tag=f"ssq{g % 2}", name=f"ssq_{g}"
                )
            mulsum(zbs[g][:, a, :], zbs[g][:, a, :], ssqs[g][:, a : a + 1])

        def prep_finish(g, two_queues=False):
            zb = zbs.pop(g)
            ssq = ssqs.pop(g)
            invn = stats.tile([P, GT], f32, tag="invn")
            rsqrt(invn[:], ssq[:])
            fnb = fnbpool.tile([P, GT, D], bf16, tag="fnb")
            tb = tbpool.tile([P, DB, R], bf16, tag="tb")
            ft8 = f8pool.tile([P, DB, R], f8, tag=f"ft8_{g}", name=f"ft8_{g}")
            for half in range(2):
                for a in range(4 * half, 4 * half + 4):
                    nc.vector.tensor_scalar_mul(
                        fnb[:, a, :], zb[:, a, :], invn[:, a : a + 1]
                    )
                    q = nc.scalar if (two_queues and a % 2 == 1) else nc.sync
                    q.dma_start(
                        out=tb[:, :, a * P : (a + 1) * P],
                        in_=fnb[:, a, :],
                        transpose=True,
                    )
                sl = slice(half * 512, half * 512 + 512)
                nc.vector.tensor_copy(ft8[:, :, sl], tb[:, :, sl])
            ft8s[g] = ft8

        def prep_group(g, two_queues=False):
            for a in range(GT):
                prep_ssq(g, a)
            prep_finish(g, two_queues)

        def sim_phase(g, prep_g=None):
            # Phases 0 and 4 are triangular: S is symmetric (g0 is the
            # diagonal block; the g4 pair-block is otherwise computed by
            # both cores of a +4 pair), so the (r>=4, ns0) quarter is
            # skipped and recovered from column sums of the computed
            # (r<4, ns1) quarter - own-core for g0, +4-partner for g4.
            ft8g = ft8s[g]
            ft80 = ft8s[0]
            tri = g in (0, 4)
            cs = cspool.tile([P, R], f32, tag="cs", name=f"cs{g}")
            e8 = None
            e8h = None
            for r in range(GT):
                ps = psum.tile([P, R], f32, tag="ps")
                ns_list = (1,) if (tri and r >= 4) else (0, 1)
                for h in range(H):
                    lhsT = ft80[:, 2 * h : 2 * h + 2, r * P : (r + 1) * P]
                    for ns in ns_list:
                        nc.tensor.matmul(
                            ps[:, ns * 512 : (ns + 1) * 512],
                            lhsT,
                            ft8g[:, 2 * h : 2 * h + 2, ns * 512 : (ns + 1) * 512],
                            start=(h == 0),
                            stop=(h == H - 1),
                            perf_mode=DR,
                        )
                if g == 0:
                    mulsum(ps[:, r * P : (r + 1) * P], eye_sb[:], diag_sb[:, r : r + 1])
                if g == 4:
                    mulsum(ps[:, r * P : (r + 1) * P], eye_sb[:], pos_sb[:, r : r + 1])
                acc = sums_sb[:, g * GT + r : g * GT + r + 1]
                if not tri:
                    if r % 2 == 0:
                        e8 = e8pool.tile([P, 2, R], f8, tag="e8")
                    nc.scalar.activation(
                        e8[:, r % 2, :], ps[:], EXPF, scale=1.0 / T, accum_out=acc
                    )
                    if r % 2 == 1:
                        pr = r // 2
                        for ns in range(2):
                            nc.tensor.matmul(
                                cs[0:1, ns * 512 : (ns + 1) * 512],
                                ones8[:, :, 0:1],
                                e8[:, :, ns * 512 : (ns + 1) * 512],
                                start=(pr == 0),
                                stop=(pr == GT // 2 - 1),
                                perf_mode=DR,
                            )
                else:
                    scr = scrpool.tile([P, R], bf16, tag="scr")
                    if r < 4:
                        nc.scalar.activation(
                            scr[:, 0:512], ps[:, 0:512], EXPF,
                            scale=1.0 / T, accum_out=acc,
                        )
                        if r % 2 == 0:
                            e8h = e8pool.tile([P, 2, 512], f8, tag="e8h")
                        ex = 40 + (0 if g == 0 else 4) + r
                        nc.scalar.activation(
                            e8h[:, r % 2, :], ps[:, 512:], EXPF,
                            scale=1.0 / T,
                            accum_out=sums_sb[:, ex : ex + 1],
                        )
                        if r % 2 == 1:
                            nc.tensor.matmul(
                                cs[0:1, 0:512],
                                ones8[:, :, 0:1],
                                e8h[:, :, :],
                                start=(r == 1),
                                stop=(r == 3),
                                perf_mode=DR,
                            )
                    else:
                        nc.scalar.activation(
                            scr[:, 0:512], ps[:, 512:], EXPF,
                            scale=1.0 / T, accum_out=acc,
                        )
                if prep_g is not None:
                    prep_ssq(prep_g, r)
            if g in (1, 2, 3):
                nc.vector.tensor_copy(
                    csum_sb[0:1, (g - 1) * R : g * R], cs[0:1, :]
                )
            else:
                off = 3 * R + (0 if g == 0 else 512)
                nc.vector.tensor_copy(
                    csum_sb[0:1, off : off + 512], cs[0:1, 0:512]
                )
            if prep_g is not None:
                prep_finish(prep_g)

        load_group(0)
        load_group(1)
        prep_group(0, two_queues=True)
        prep_group(1, two_queues=True)
        for g in range(NG):
            if g + 2 < NG:
                load_group(g + 2)
            sim_phase(g, prep_g=g + 2 if g + 2 < NG else None)

        nc.sync.dma_start(out=sums_out[:], in_=sums_sb[:])
        nc.sync.dma_start(out=diag_out[:], in_=diag_sb[:])
        nc.sync.dma_start(out=pos_out[:], in_=pos_sb[:])
        nc.sync.dma_start(out=csum_out[:, :], in_=csum_sb[0:1, :])

    nc.compile()
    return nc


def _get_nc():
    global _NC
    if _NC is None:
        _NC = _build()
    return _NC


def run(z1, z2, trace=False):
    from concourse.bass_utils import run_bass_kernel_spmd

    z1 = np.ascontiguousarray(z1, dtype=np.float32)
    z2 = np.ascontiguousarray(z2, dtype=np.float32)
    F = np.concatenate([z1, z2], axis=0)
    eye_np = np.eye(P, dtype=np.float32)
    in_maps = []
    for c in range(NCORES):
        m = {"eye": eye_np}
        for k in range(NG):
            blk = (c + k) % G
            m[f"f{k}"] = F[blk * R : (blk + 1) * R]
        in_maps.append(m)
    res = run_bass_kernel_spmd(
        _get_nc(), in_maps, core_ids=list(range(NCORES)), trace=trace
    )
    e_diag_true = np.exp(1.0 / T)
    RS, DG, PS, CSa, CS0, CS4 = [], [], [], [], [], []
    for r in res.results:
        sums = r["sums"].astype(np.float64)
        base = sums[:, :40].reshape(P, NG, GT).sum(axis=1)
        # extra columns: ns1-half rowsums of the split (r<4) exps of the
        # triangular phases 0 and 4
        base[:, 0:4] += sums[:, 40:44] + sums[:, 44:48]
        RS.append(base.T.reshape(R))
        DG.append(r["diag"].astype(np.float64).T.reshape(R))
        PS.append(r["pos"].astype(np.float64).T.reshape(R))
        cs = r["csum"].astype(np.float64)
        CSa.append(cs[0, : 3 * R].reshape(3, R))
        CS0.append(cs[0, 3 * R : 3 * R + 512])
        CS4.append(cs[0, 3 * R + 512 :])
    total = 0.0
    for b in range(G):
        den = RS[b] - np.exp(DG[b] / T) + e_diag_true
        for g in (1, 2, 3):
            den = den + CSa[(b - g) % G][g - 1]
        # triangular recovery: rows 512+ get their (cols < 512)
        # contributions from the g0 (own core) and g4 (+4 partner)
        # transpose column sums
        den[512:] = den[512:] + CS0[b] + CS4[(b + 4) % G]
        total += (np.log(den) - PS[b] / T).sum()
    loss = total / (2.0 * B)
    return np.float32(loss), res


def kernel(z1, z2, labels=None, **_ignored):
    loss, _ = run(z1, z2, trace=False)
    return np.asarray(loss, dtype=np.float32)


if __name__ == "__main__":
    rng = np.random.default_rng(0)
    a = rng.standard_normal((B, D)).astype(np.float32)
    b = rng.standard_normal((B, D)).astype(np.float32)
    print(kernel(a, b, None))


# revision 60
# speedup vs baseline: 1.0210x; 1.0210x over previous
"""Trainium2 Bass kernel for nn_ContrastiveLoss (SimCLR-style NT-Xent).

Reference computation:
    f = normalize(concat([z1, z2]))            # [2B, D] unit rows
    S = f @ f.T / T                            # [8192, 8192]
    loss = mean_i( logsumexp_j(S[i, :]) - S[i, pos_i] )

Symmetric sharding: S is symmetric, so each of the 8 cores computes only
5 of the 8 column-groups of its 1024-row block (groups 0..4 after
rotating the row-groups so the core's own rows are group 0).  The
missing column groups 5,6,7 of row-block b are transposes of blocks
computed on cores b-3..b-1 and are recovered as COLUMN sums of the
exp'd blocks g=1..3 (a [128, 2, 512] fp8 DoubleRow ones-matmul per
row-tile pair, accumulated in PSUM), exchanged between cores by the
host during the final cheap f64 reduction.  This cuts matmul + exp work
to 5/8 and HBM traffic to 10 MB/core.  On top of that, phases 0 and 4
are TRIANGULAR: g0 is the symmetric diagonal block and the g4 pair
block is otherwise computed by both cores of a +4 pair, so the
(r>=4, ns0) quarter of each is skipped and recovered from column sums
of the computed (r<4, ns1) quarter (own-core for g0, +4-partner for
g4; the r<4 exps split into a bf16-scratch ns0 half and an fp8 ns1
half feeding a [128, 2, 512] DR ones-matmul, with the ns1 row-sums
landing in 8 extra sums columns).

Operand layout: rows are normalized in row-major bf16 (DVE
affine_mul_reduce sum-of-squares + Quake rsqrt + scale), DMA-xbar
transposed as native 2-byte elements into [dp, db, col] (d = 128*db +
dp, one transpose per row-tile to stay within the 2D-in/3D-out xbar
constraint), then cast to fp8e4 per column half.  A DoubleRow
contraction pair (dp, t) maps to d = 256h + 128t + dp, so BOTH operands
slice straight out of the same [128, 4, 1024] fp8 tile with far-strided
(1024B) k-pairs and contiguous columns - the layout the double-pumped
weight/ifmap streams require (byte-interleaved pairs run 1 elem/cycle).

Pipeline (measured-best arrangement): loads are SWDGE f32->bf16
cast-DMAs (~150 GB/s each), paced two-wide by chaining chunk n behind
chunk n-2; group g+2's ssq mul-reduces are drip-fed one per row-tile
through phase g's DVE queue, and its rsqrt/scale+transpose/cast tail is
emitted at phase end; startup transposes for groups 0/1 are split over
both HWDGE queues (SP + ACT) while ACT is idle.  Per row-tile r, phase
g: 4 DR matmuls -> [128, 1024] psum; diag (g=0) / pos-pair (g=4) raw
cosines are extracted pre-exp from PSUM with an eye mul-reduce; ACT
exps the block with a fused row-sum (accum_out), writing bf16 scratch
(g=0/4) or fp8 for the colsum matmuls (g=1..3).
NOTE: tensor_tensor_reduce hangs TRN2 hardware (sim is fine) - all
mul-reduces must use affine_mul_reduce.

Host (f64) assembles denominators across cores:
  den[b] = rowsums_b - exp(diag_b/T) + e^{1/T} + sum_g colsums_{b-g}[g]
  loss   = mean(log(den) - pos/T)
The exact-diagonal substitution cancels the fp8 quantization noise of
the dominant e^{1/T} ~ 1.6e6 softmax term (the rest of a row sums to
~1e4).  Off-diagonal cosines are within ~+-0.25 whp, so exp(S/T) fits
fp8e4 directly for the colsum operands.  No logsumexp max-subtraction
is needed: sum_j exp() <= ~2e10 fits fp32.

Measured on TRN2 (8 cores): 132909-133434 ns in a heat-degraded
session window where the pre-triangular config measured 147-155 us
(fresh-device best of that config: 130352 ns), vs the 207-222 us v1
baseline; rel err 4.1e-5.  Alternatives measured
SLOWER and reverted: load lookahead-3 with up-front prep blobs (+8us),
scale-on-ACT (+10), per-phase output DMAs (+20), hard cross-group DVE
ordering deps (+15), ns-major warm-start matmuls (+20 combined), ssq
via ACT Square (+7, though it improves rel err to 7e-6).
"""

import os
import sys

# Reset the NeuronCores when the runtime opens the device: a prior run
# leaving the part in a degraded power/exec state costs 10-15% measured
# exec time (observed 133 -> 148 us on identical binaries); the reset
# restores the fast window.  setdefault so an explicit env still wins.
os.environ.setdefault("NEURON_RT_RESET_CORES", "1")

try:
    import concourse.bass  # noqa: F401
except ImportError:
    for _p in ("/root/.axon_site/_ro/trn_rl_repo", "/opt/trn_rl_repo"):
        if _p not in sys.path and os.path.isdir(_p):
            sys.path.insert(0, _p)

import numpy as np

B = 4096
D = 512
T = 0.07
P = 128
NCORES = 8
R = (2 * B) // NCORES
G = 8
NG = 5
GT = R // P
H = 2
DB = D // P

_NC = None


def _build():
    from contextlib import ExitStack

    import concourse.bacc as bacc
    import concourse.tile as tile
    from concourse import mybir
    from concourse.tile import add_dep_helper

    f32 = mybir.dt.float32
    bf16 = mybir.dt.bfloat16
    f8 = mybir.dt.float8e4
    i32 = mybir.dt.int32
    AFT = mybir.ActivationFunctionType
    EXPF = AFT.Exp
    MUL = mybir.AluOpType.mult
    ADD = mybir.AluOpType.add
    SUB = mybir.AluOpType.subtract
    SHR = mybir.AluOpType.logical_shift_right
    DR = mybir.MatmulPerfMode.DoubleRow

    nc = bacc.Bacc(
        "TRN2", target_bir_lowering=False, debug=False, num_devices=NCORES
    )
    fg = [
        nc.dram_tensor(f"f{k}", [R, D], f32, kind="ExternalInput")
        for k in range(NG)
    ]
    eye = nc.dram_tensor("eye", [P, P], f32, kind="ExternalInput")
    sums_out = nc.dram_tensor("sums", [P, NG * GT + 8], f32, kind="ExternalOutput")
    diag_out = nc.dram_tensor("diag", [P, GT], f32, kind="ExternalOutput")
    pos_out = nc.dram_tensor("pos", [P, GT], f32, kind="ExternalOutput")
    csum_out = nc.dram_tensor("csum", [1, 3 * R + 1024], f32, kind="ExternalOutput")

    with ExitStack() as ctx:
        tc = ctx.enter_context(tile.TileContext(nc))
        smalls = ctx.enter_context(tc.tile_pool(name="smalls", bufs=1))
        dumps = ctx.enter_context(tc.tile_pool(name="dumps", bufs=4))
        stats = ctx.enter_context(tc.tile_pool(name="stats", bufs=3))
        zbpool = ctx.enter_context(tc.tile_pool(name="zbpool", bufs=3))
        tbpool = ctx.enter_context(tc.tile_pool(name="tbpool", bufs=2))
        f8pool = ctx.enter_context(tc.tile_pool(name="f8pool", bufs=1))
        e8pool = ctx.enter_context(tc.tile_pool(name="e8pool", bufs=2))
        scrpool = ctx.enter_context(tc.tile_pool(name="scrpool", bufs=2))
        psum = ctx.enter_context(tc.tile_pool(name="psum", bufs=3, space="PSUM"))
        cspool = ctx.enter_context(tc.tile_pool(name="cspool", bufs=1, space="PSUM"))

        sums_sb = smalls.tile([P, NG * GT + 8], f32, tag="sums_sb")
        diag_sb = smalls.tile([P, GT], f32, tag="diag_sb")
        pos_sb = smalls.tile([P, GT], f32, tag="pos_sb")
        csum_sb = smalls.tile([1, 3 * R + 1024], f32, tag="csum_sb")
        eye_sb = smalls.tile([P, P], f32, tag="eye_sb")
        nc.sync.dma_start(out=eye_sb[:], in_=eye[:, :])
        magic = smalls.tile([P, GT], i32, tag="magic")
        nc.vector.memset(magic[:], 0x5F3759DF)
        ones8 = smalls.tile([P, 2, 16], f8, tag="ones8")
        nc.vector.memset(ones8[:], 1.0)

        def mulsum(in0, in1, accum_col):
            dummy = dumps.tile([P, 1], f32, tag="dummy")
            return nc.vector.affine_mul_reduce(
                out=dummy.broadcast_to(in0.shape),
                accum_out=accum_col,
                in0=in0,
                in1=in1,
                scale=1.0,
                bias=0.0,
            )

        def rsqrt(invn_dst, ssq):
            n = ssq.shape[1]
            h = stats.tile([P, n], i32, tag="h")
            nc.vector.tensor_scalar(h[:], ssq.bitcast(i32), 1, None, op0=SHR)
            y = stats.tile([P, n], f32, tag="y")
            nc.vector.tensor_tensor(y[:].bitcast(i32), magic[:, :n], h[:], op=SUB)
            a = stats.tile([P, n], f32, tag="a")
            for _ in range(2):
                nc.vector.tensor_mul(a[:], y[:], y[:])
                nc.vector.tensor_mul(a[:], a[:], ssq)
                nc.vector.tensor_scalar(a[:], a[:], -0.5, 1.5, op0=MUL, op1=ADD)
                nc.vector.tensor_mul(y[:], y[:], a[:])
            nc.vector.tensor_scalar_min(invn_dst, y[:], 1.0e12)

        load_insts = []
        zbs = {}

        def load_group(g):
            zb = zbpool.tile([P, GT, D], bf16, tag="zb")
            for s in range(2):
                ld = nc.gpsimd.dma_start(
                    out=zb[:, s * 4 : (s + 1) * 4, :],
                    in_=fg[g][s * 4 * P : (s + 1) * 4 * P, :].rearrange(
                        "(a p) d -> p a d", p=P
                    ),
                )
                n = len(load_insts)
                if n >= 2:
                    add_dep_helper(
                        ld.ins, load_insts[n - 2].ins, reason="pace loads"
                    )
                load_insts.append(ld)
            zbs[g] = zb

        ft8s = {}
        ssqs = {}

        def prep_ssq(g, a):
            if g not in ssqs:
                ssqs[g] = stats.tile(
                    [P, GT], f32, tag=f"ssq{g % 2}", name=f"ssq_{g}"
                )
            mulsum(zbs[g][:, a, :], zbs[g][:, a, :], ssqs[g][:, a : a + 1])

        def prep_finish(g, two_queues=False):
            zb = zbs.pop(g)
            ssq = ssqs.pop(g)
            invn = stats.tile([P, GT], f32, tag="invn")
            rsqrt(invn[:], ssq[:])
            tb = tbpool.tile([P, DB, R], bf16, tag="tb")
            ft8 = f8pool.tile([P, DB, R], f8, tag=f"ft8_{g}", name=f"ft8_{g}")
            for half in range(2):
                for a in range(4 * half, 4 * half + 4):
                    nc.vector.tensor_scalar_mul(
                        zb[:, a, :], zb[:, a, :], invn[:, a : a + 1]
                    )
                    q = nc.scalar if (two_queues and a % 2 == 1) else nc.sync
                    q.dma_start(
                        out=tb[:, :, a * P : (a + 1) * P],
                        in_=zb[:, a, :],
                        transpose=True,
                    )
                sl = slice(half * 512, half * 512 + 512)
                nc.vector.tensor_copy(ft8[:, :, sl], tb[:, :, sl])
            ft8s[g] = ft8

        def prep_group(g, two_queues=False):
            for a in range(GT):
                prep_ssq(g, a)
            prep_finish(g, two_queues)

        def sim_phase(g, prep_g=None):
            # Phases 0 and 4 are triangular: S is symmetric (g0 is the
            # diagonal block; the g4 pair-block is otherwise computed by
            # both cores of a +4 pair), so the (r>=4, ns0) quarter is
            # skipped and recovered from column sums of the computed
            # (r<4, ns1) quarter - own-core for g0, +4-partner for g4.
            ft8g = ft8s[g]
            ft80 = ft8s[0]
            tri = g in (0, 4)
            cs = cspool.tile([P, R], f32, tag="cs", name=f"cs{g}")
            e8 = None
            e8h = None
            for r in range(GT):
                ps = psum.tile([P, R], f32, tag="ps")
                ns_list = (1,) if (tri and r >= 4) else (0, 1)
                for h in range(H):
                    lhsT = ft80[:, 2 * h : 2 * h + 2, r * P : (r + 1) * P]
                    for ns in ns_list:
                        nc.tensor.matmul(
                            ps[:, ns * 512 : (ns + 1) * 512],
                            lhsT,
                            ft8g[:, 2 * h : 2 * h + 2, ns * 512 : (ns + 1) * 512],
                            start=(h == 0),
                            stop=(h == H - 1),
                            perf_mode=DR,
                        )
                if g == 0:
                    mulsum(ps[:, r * P : (r + 1) * P], eye_sb[:], diag_sb[:, r : r + 1])
                if g == 4:
                    mulsum(ps[:, r * P : (r + 1) * P], eye_sb[:], pos_sb[:, r : r + 1])
                acc = sums_sb[:, g * GT + r : g * GT + r + 1]
                if not tri:
                    if r % 2 == 0:
                        e8 = e8pool.tile([P, 2, R], f8, tag="e8")
                    nc.scalar.activation(
                        e8[:, r % 2, :], ps[:], EXPF, scale=1.0 / T, accum_out=acc
                    )
                    if r % 2 == 1:
                        pr = r // 2
                        for ns in range(2):
                            nc.tensor.matmul(
                                cs[0:1, ns * 512 : (ns + 1) * 512],
                                ones8[:, :, 0:1],
                                e8[:, :, ns * 512 : (ns + 1) * 512],
                                start=(pr == 0),
                                stop=(pr == GT // 2 - 1),
                                perf_mode=DR,
                            )
                else:
                    scr = scrpool.tile([P, R], bf16, tag="scr")
                    if r < 4:
                        nc.scalar.activation(
                            scr[:, 0:512], ps[:, 0:512], EXPF,
                            scale=1.0 / T, accum_out=acc,
                        )
                        if r % 2 == 0:
                            e8h = e8pool.tile([P, 2, 512], f8, tag="e8h")
                        ex = 40 + (0 if g == 0 else 4) + r
                        nc.scalar.activation(
                            e8h[:, r % 2, :], ps[:, 512:], EXPF,
                            scale=1.0 / T,
                            accum_out=sums_sb[:, ex : ex + 1],
                        )
                        if r % 2 == 1:
                            nc.tensor.matmul(
                                cs[0:1, 0:512],
                                ones8[:, :, 0:1],
                                e8h[:, :, :],
                                start=(r == 1),
                                stop=(r == 3),
                                perf_mode=DR,
                            )
                    else:
                        nc.scalar.activation(
                            scr[:, 0:512], ps[:, 512:], EXPF,
                            scale=1.0 / T, accum_out=acc,
                        )
                if prep_g is not None:
                    prep_ssq(prep_g, r)
            if g in (1, 2, 3):
                nc.vector.tensor_copy(
                    csum_sb[0:1, (g - 1) * R : g * R], cs[0:1, :]
                )
            else:
                off = 3 * R + (0 if g == 0 else 512)
                nc.vector.tensor_copy(
                    csum_sb[0:1, off : off + 512], cs[0:1, 0:512]
                )
            if prep_g is not None:
                prep_finish(prep_g)

        load_group(0)
        load_group(1)
        prep_group(0, two_queues=True)
        prep_group(1, two_queues=True)
        for g in range(NG):
            if g + 2 < NG:
                load_group(g + 2)
            sim_phase(g, prep_g=g + 2 if g + 2 < NG else None)

        nc.sync.dma_start(out=sums_out[:], in_=sums_sb[:])
        nc.sync.dma_start(out=diag_out[:], in_=diag_sb[:])
        nc.sync.dma_start(out=pos_out[:], in_=pos_sb[:])
        nc.sync.dma_start(out=csum_out[:, :], in_=csum_sb[0:1, :])

    nc.compile()
    return nc


def _get_nc():
    global _NC
    if _NC is None:
        _NC = _build()
    return _NC


def run(z1, z2, trace=False):
    from concourse.bass_utils import run_bass_kernel_spmd

    z1 = np.ascontiguousarray(z1, dtype=np.float32)
    z2 = np.ascontiguousarray(z2, dtype=np.float32)
    F = np.concatenate([z1, z2], axis=0)
    eye_np = np.eye(P, dtype=np.float32)
    in_maps = []
    for c in range(NCORES):
        m = {"eye": eye_np}
        for k in range(NG):
            blk = (c + k) % G
            m[f"f{k}"] = F[blk * R : (blk + 1) * R]
        in_maps.append(m)
    res = run_bass_kernel_spmd(
        _get_nc(), in_maps, core_ids=list(range(NCORES)), trace=trace
    )
    e_diag_true = np.exp(1.0 / T)
    RS, DG, PS, CSa, CS0, CS4 = [], [], [], [], [], []
    for r in res.results:
        sums = r["sums"].astype(np.float64)
        base = sums[:, :40].reshape(P, NG, GT).sum(axis=1)
        # extra columns: ns1-half rowsums of the split (r<4) exps of the
        # triangular phases 0 and 4
        base[:, 0:4] += sums[:, 40:44] + sums[:, 44:48]
        RS.append(base.T.reshape(R))
        DG.append(r["diag"].astype(np.float64).T.reshape(R))
        PS.append(r["pos"].astype(np.float64).T.reshape(R))
        cs = r["csum"].astype(np.float64)
        CSa.append(cs[0, : 3 * R].reshape(3, R))
        CS0.append(cs[0, 3 * R : 3 * R + 512])
        CS4.append(cs[0, 3 * R + 512 :])
    total = 0.0
    for b in range(G):
        den = RS[b] - np.exp(DG[b] / T) + e_diag_true
        for g in (1, 2, 3):
            den = den + CSa[(b - g) % G][g - 1]
        # triangular recovery: rows 512+ get their (cols < 512)
        # contributions from the g0 (own core) and g4 (+4 partner)
        # transpose column sums
        den[512:] = den[512:] + CS0[b] + CS4[(b + 4) % G]
        total += (np.log(den) - PS[b] / T).sum()
    loss = total / (2.0 * B)
    return np.float32(loss), res


def kernel(z1, z2, labels=None, **_ignored):
    loss, _ = run(z1, z2, trace=False)
    return np.asarray(loss, dtype=np.float32)


if __name__ == "__main__":
    rng = np.random.default_rng(0)
    a = rng.standard_normal((B, D)).astype(np.float32)
    b = rng.standard_normal((B, D)).astype(np.float32)
    print(kernel(a, b, None))
